# revision 40
# baseline (speedup 1.0000x reference)
"""DeBERTa disentangled-attention block on 8 Trainium2 NeuronCores.

Sharding: core c owns batch b = c//2 and heads [6*(c%2), 6*(c%2)+6).
Per core the device computes q/k/v and positional projections, transposed
attention scores scoresT[t,s] = k.q_s + c2p + p2c with the two
relative-position biases rendered via GPSIMD shared-index expansion along
diagonal windows + per-partition-offset SBUF->SBUF shift DMAs, fused into the
scores PSUM by PE transpose-accumulate (c2p, fp32) and identity-matmul (p2c,
bf16). exp() runs unnormalized (Z comes from a fused ones-column in the ctx
matmul); probsT and Z stream to HBM and the host finishes the transpose and
1/Z normalization. The output projection partials are AllReduced pairwise
across cores, then residual + LayerNorm on device.

The host side only reshapes/shards inputs and reassembles outputs.
"""

import numpy as np

import concourse.mybir as mybir
import concourse.tile as tile
from concourse.tile import add_dep_helper
from concourse import bacc
from concourse import bass_utils

F32 = mybir.dt.float32
F32R = mybir.dt.float32r
BF16 = mybir.dt.bfloat16
U16 = mybir.dt.uint16

HEADS = 12
B, S, H, D = 4, 1024, 768, 64
SPAN = 256
NB = 512
MAXPOS = 512
LN_EPS = 1e-7
SCALE = float(1.0 / np.sqrt(np.float32(D) * 3.0))
NH = 6
KCH = [128] * 6 + [1]
W_I = [896 - 128 * i for i in range(8)]
MU_COLS = 64 + 8  # wrapped cols per block: 1024-gather + 128-gather
DEBUG_DUMPS = False
SINGLE_CORE_TIMING = False


def _make_log_bucket_position(rel, bucket_size=SPAN, max_position=MAXPOS):
    mid = bucket_size // 2
    sign = np.sign(rel)
    abs_pos = np.where((rel < mid) & (rel > -mid), mid - 1, np.abs(rel)).astype(
        np.float32
    )
    log_pos = (
        np.ceil(np.log(abs_pos / mid) / np.log((max_position - 1) / mid) * (mid - 1))
        + mid
    )
    return np.where(abs_pos <= mid, rel, (log_pos * sign).astype(np.int64)).astype(
        np.int64
    )


def _mu_tables():
    j = np.arange(2048)
    bucket = _make_log_bucket_position(1023 - j)
    mu1 = np.clip(bucket + SPAN, 0, NB - 1).astype(np.uint16)
    mu2 = np.clip(-bucket + SPAN, 0, NB - 1).astype(np.uint16)
    return mu1, mu2


def _wrap_list(win):
    """indirect_copy wrapped layout: unwrapped[x] = wrapped[x % 16, x // 16],
    replicated across the 8 16-partition groups."""
    n = len(win)
    w = np.zeros((16, n // 16), dtype=np.uint16)
    x = np.arange(n)
    w[x % 16, x // 16] = win
    return np.tile(w, (8, 1))


def _plan_segments(mu):
    """Per block: decompose the 1152-wide diagonal window into
    bcast-prefix / gather / affine / gather / bcast-suffix segments.

    Returns (plans, mu_w): plans[i] = list of (kind, x0, x1, prm) with kinds
    'g' (prm = wrapped col offset), 'aff' (prm = (src_col_at_x0, step)),
    'b' (prm = src col); mu_w = [128, total_wrapped_cols] uint16.
    """
    plans = []
    wraps = []
    col_ofs = 0
    n = 1152
    for i in range(8):
        win = mu[W_I[i] : W_I[i] + n].astype(int)
        p = 1
        while p < n and win[p] == win[0]:
            p += 1
        s = n - 1
        while s > 0 and win[s - 1] == win[n - 1]:
            s -= 1
        xc = 1023 - W_I[i]
        step = int(win[xc + 1] - win[xc]) if xc + 1 < n else 1
        assert step in (-1, 1)
        a0 = xc
        while a0 > 0 and win[a0 - 1] == win[a0] - step:
            a0 -= 1
        a1 = xc
        while a1 + 1 < n and win[a1 + 1] == win[a1] + step:
            a1 += 1
        a1 += 1
        pref_end = p if p >= 32 else 0
        suff_start = s if (n - s) >= 32 else n
        a0 = max(a0, pref_end)
        a1 = min(a1, suff_start)
        if a1 < a0:
            a0 = a1 = max(pref_end, min(a0, suff_start))
        # gather zones, widths rounded up to 16 by eating into neighbors
        g1s, g1e = pref_end, a0
        w1 = g1e - g1s
        if w1 % 16:
            g1e += 16 - (w1 % 16)
            a0 = g1e
        g2s, g2e = a1, suff_start
        w2 = g2e - g2s
        if w2 % 16:
            g2e += 16 - (w2 % 16)
            if g2e > n:
                d = g2e - n
                g2s -= d
                g2e = n
                a1 = min(a1, g2s)
        if a0 > a1:  # affine squeezed out; merge into gather 2
            g2s = g1e
            a0 = a1 = g1e
        segs = []
        if pref_end > 0:
            segs.append(("b", 0, pref_end, int(win[0])))
        if g1e > g1s:
            segs.append(("g", g1s, g1e, col_ofs))
            wraps.append(_wrap_list(win[g1s:g1e].astype(np.uint16)))
            col_ofs += (g1e - g1s) // 16
        if a1 > a0:
            segs.append(("aff", a0, a1, (int(win[a0]), step)))
        if g2e > g2s:
            segs.append(("g", g2s, g2e, col_ofs))
            wraps.append(_wrap_list(win[g2s:g2e].astype(np.uint16)))
            col_ofs += (g2e - g2s) // 16
        if g2e < n:
            segs.append(("b", g2e, n, int(win[n - 1])))
        # verify reconstruction
        chk = np.zeros(n, dtype=int) - 1
        for kind, x0, x1, prm in segs:
            if kind == "b":
                chk[x0:x1] = prm
            elif kind == "aff":
                c0, st = prm
                chk[x0:x1] = c0 + st * np.arange(x1 - x0)
            else:
                chk[x0:x1] = win[x0:x1]
        assert (chk == win).all(), f"segment plan broken for block {i}"
        plans.append(segs)
    mu_w = (
        np.concatenate(wraps, axis=1)
        if wraps
        else np.zeros((128, 16), np.uint16)
    )
    if mu_w.shape[1] % 16:
        pad = 16 - mu_w.shape[1] % 16
        mu_w = np.concatenate([mu_w, np.zeros((128, pad), np.uint16)], axis=1)
    return plans, mu_w


_MU1, _MU2 = _mu_tables()
_PLANS1, _MU1W = _plan_segments(_MU1)
_PLANS2, _MU2W = _plan_segments(_MU2)


def _expand_window(nc, plans, blk, out_tile, att_ap, mu_tile, rev_engine, b_engine):
    """Render one 1152-wide diagonal window from a [128, 512] bucket table.

    Returns the list of writer instructions (for explicit dep management:
    the downstream shifted-AP DMA read is not footprint-tracked reliably).
    """
    att_base = att_ap.offset
    pitch = att_ap.ap[0][0]
    writers = []
    for kind, x0, x1, prm in plans[blk]:
        w = x1 - x0
        if kind == "g":
            ins = nc.gpsimd.indirect_copy(
                out=out_tile[:, x0:x1],
                data=att_ap,
                idxs=mu_tile[:, prm : prm + w // 16],
                i_know_ap_gather_is_preferred=True,
            )
        elif kind == "aff":
            c0, step = prm
            srcv = att_ap.__replace__(
                offset=att_base + c0, ap=[[pitch, 128], [step, w]]
            )
            ins = rev_engine(out_tile[:, x0:x1], srcv)
        else:  # bcast
            srcv = att_ap.__replace__(
                offset=att_base + prm, ap=[[pitch, 128], [0, w]]
            )
            ins = b_engine(out_tile[:, x0:x1], srcv)
        writers.append(ins)
    return writers


def _emit(tc, t):
    nc = tc.nc
    AF = mybir.ActivationFunctionType
    OP = mybir.AluOpType

    with (
        tc.tile_pool(name="persist", bufs=1) as pp,
        tc.tile_pool(name="qkv", bufs=1) as qp,
    ):
        identf = pp.tile([128, 128], F32)
        nc.sync.dma_start(identf[:], t["ident_f32"][:])
        identb = pp.tile([128, 128], BF16)
        nc.scalar.copy(identb[:], identf[:])
        mu1 = pp.tile([128, _MU1W.shape[1]], U16)
        mu2 = pp.tile([128, _MU2W.shape[1]], U16)
        nc.sync.dma_start(mu1[:], t["mu1_w"][:])
        nc.sync.dma_start(mu2[:], t["mu2_w"][:])

        qTs = [qp.tile([128, 1024], F32R, name=f"qTs{i}", tag=f"qTs{i}") for i in range(3)]
        kT = [qp.tile([128, 1024], F32R, name=f"kT{i}", tag=f"kT{i}") for i in range(3)]
        PkX = [qp.tile([128, 512], F32R, name=f"pkx{i}", tag=f"pkx{i}") for i in range(3)]
        PqXs = [qp.tile([128, 512], F32R, name=f"pqx{i}", tag=f"pqx{i}") for i in range(3)]
        vaug = [qp.tile([128, 390], F32R, name=f"vaug{i}", tag=f"vaug{i}") for i in range(8)]
        ctxT = [qp.tile([128, 1024], F32R, name=f"ctxT{i}", tag=f"ctxT{i}") for i in range(3)]

        with (
            tc.tile_pool(name="wload", bufs=1) as wp,
            tc.tile_pool(name="psP", bufs=2, space="PSUM") as psA,
        ):
            def load_rounded(name, cols):
                # host packs [769, cols] as [128, 7*cols]; chunk c at col
                # block c. DMA raw fp32 bits into the f32r tile (bitcast),
                # then round in place so no separate raw buffer is needed.
                rnd = wp.tile(
                    [128, 7 * cols], F32R, name=f"{name}r_pk", tag=f"{name}r"
                )
                nc.sync.dma_start(rnd[:], t[name][:].bitcast(F32R))
                nc.vector.tensor_copy(rnd[:], rnd[:].bitcast(F32))
                tiles = []
                for ci, kc in enumerate(KCH):
                    tiles.append(rnd[0:kc, ci * cols : (ci + 1) * cols])
                return tiles

            xT = load_rounded("xT_aug", 1024)
            wq = load_rounded("WqTs_aug", 384)
            wk = load_rounded("WkT_aug", 384)
            wv = load_rounded("WvT_aug", 384)
            pos = load_rounded("pos_embT_aug", 512)

            bqc = pp.tile([128, 3], F32, name="bq_c", tag="bq_c")
            nc.sync.dma_start(bqc[:], t["bq_c"][:])
            bkc = pp.tile([128, 3], F32, name="bk_c", tag="bk_c")
            nc.sync.dma_start(bkc[:], t["bk_c"][:])
            for dst, w, bcol in ((qTs, wq, bqc), (kT, wk, bkc)):
                for mb in range(3):
                    for ns in range(2):
                        ps = psA.tile([128, 512], F32, tag="psproj")
                        for ci in range(6):
                            nc.tensor.matmul(
                                ps[:],
                                w[ci][:, mb * 128 : (mb + 1) * 128],
                                xT[ci][:, ns * 512 : (ns + 1) * 512],
                                start=(ci == 0),
                                stop=(ci == 5),
                            )
                        nc.scalar.activation(
                            dst[mb][:, ns * 512 : (ns + 1) * 512],
                            ps[:],
                            AF.Identity,
                            bias=bcol[:, mb : mb + 1],
                        )

            for dst, w, bcol in ((PkX, wk, bkc), (PqXs, wq, bqc)):
                for mb in range(3):
                    ps = psA.tile([128, 512], F32, tag="psproj")
                    for ci in range(6):
                        nc.tensor.matmul(
                            ps[:],
                            w[ci][:, mb * 128 : (mb + 1) * 128],
                            pos[ci][:],
                            start=(ci == 0),
                            stop=(ci == 5),
                        )
                    nc.scalar.activation(
                        dst[mb][:], ps[:], AF.Identity, bias=bcol[:, mb : mb + 1]
                    )

            for tb in range(8):
                ps = psA.tile([128, 512], F32, tag="psproj")
                for ci in range(len(KCH)):
                    nc.tensor.matmul(
                        ps[:, 0:384],
                        xT[ci][:, tb * 128 : (tb + 1) * 128],
                        wv[ci][:],
                        start=(ci == 0),
                        stop=(ci == len(KCH) - 1),
                    )
                vv = vaug[tb][:]
                vout = vv.__replace__(
                    offset=vv.offset, ap=[[390, 128], [65, 6], [1, 64]]
                )
                pv = ps[:]
                vin = pv.__replace__(
                    offset=pv.offset, ap=[[pv.ap[0][0], 128], [64, 6], [1, 64]]
                )
                nc.scalar.copy(vout, vin)
                ones_view = vaug[tb][:].__replace__(
                    offset=vaug[tb][:].offset + 64, ap=[[390, 128], [65, 6]]
                )
                nc.scalar.activation(
                    ones_view, identf[:, 0:6], AF.Copy, bias=1.0, scale=0.0
                )

        woT = []
        for kc3 in range(3):
            raw = pp.tile([128, 768], F32, name=f"wo_{kc3}", tag="woraw", bufs=2)
            nc.sync.dma_start(raw[:], t["WoT_slice"][kc3 * 128 : (kc3 + 1) * 128, :])
            rnd = pp.tile([128, 768], F32R, name=f"wor_{kc3}", tag=f"wor_{kc3}")
            nc.scalar.copy(rnd[:], raw[:])
            woT.append(rnd)

        # ---------------- attention units ----------------
        with (
            tc.tile_pool(name="att", bufs=3) as ap_,
            tc.tile_pool(name="bc", bufs=4) as bcp,
            tc.tile_pool(name="bp", bufs=1) as bpp,
            tc.tile_pool(name="c2p", bufs=1) as cpp,
            tc.tile_pool(name="p2", bufs=1) as p2p,
            tc.tile_pool(name="ee", bufs=7) as ep,
            tc.tile_pool(name="small", bufs=2) as smp,
            tc.tile_pool(name="psS", bufs=3, space="PSUM") as psS,
            tc.tile_pool(name="psC", bufs=2, space="PSUM") as psC,
            tc.tile_pool(name="psA", bufs=2, space="PSUM") as psA,
        ):
            bc_hist = []
            bp_hist = [[] for _ in range(8)]
            bp_gathers = [None] * 8
            for h in range(NH):
                hb, hr = h // 2, (h % 2) * 64
                bp_tiles = [None] * 8
                p2_tiles = [None] * 8
                for half in range(2):
                    s0 = half * 512
                    c2p_t = []
                    for sbl in range(4):
                        sb = half * 4 + sbl
                        ps = psA.tile([128, 512], F32, tag="psatt")
                        nc.tensor.matmul(
                            ps[:],
                            qTs[hb][hr : hr + 64, sb * 128 : (sb + 1) * 128],
                            PkX[hb][hr : hr + 64, :],
                            start=True,
                            stop=True,
                        )
                        attc = ap_.tile([128, 512], BF16, tag="attc")
                        if sbl % 2 == 0:
                            nc.scalar.copy(attc[:], ps[:])
                        else:
                            nc.vector.tensor_copy(attc[:], ps[:])
                        bc = bcp.tile([128, 1152], BF16, tag="bc")
                        writers = _expand_window(
                            nc, _PLANS1, sb, bc, attc[:], mu1,
                            nc.vector.tensor_copy, nc.scalar.copy,
                        )
                        # WAR: this slot's previous shifted read must finish
                        prev = bc_hist[-4] if len(bc_hist) >= 4 else None
                        if prev is not None:
                            for wri in writers:
                                add_dep_helper(wri.ins, prev.ins, sync=True, reason="bc WAR")
                        ct = cpp.tile([128, 1024], BF16, name=f"c2p{sbl}", tag=f"c2p{sbl}", bufs=2)
                        bv = bc[:]
                        sh = nc.sync.dma_start(
                            ct[:],
                            bv.__replace__(
                                offset=bv.offset + 127, ap=[[1151, 128], [1, 1024]]
                            ),
                        )
                        # RAW: the diagonal-view read must wait for all writers
                        for wri in writers:
                            add_dep_helper(sh.ins, wri.ins, sync=True, reason="bc RAW")
                        bc_hist.append(sh)
                        if DEBUG_DUMPS and h == NH - 1:
                            nc.sync.dma_start(t["attc_dbg"][sb], attc[:])
                            nc.sync.dma_start(t["bc_dbg"][sb], bc[:])
                            nc.sync.dma_start(t["c2p_dbg"][sb], ct[:])
                        c2p_t.append(ct)

                    ps_ctx = psC.tile([65, 512], F32, tag="psctx")
                    for tb in range(8):
                        if half == 0:
                            ps = psA.tile([128, 512], F32, tag="psatt")
                            nc.tensor.matmul(
                                ps[:],
                                kT[hb][hr : hr + 64, tb * 128 : (tb + 1) * 128],
                                PqXs[hb][hr : hr + 64, :],
                                start=True,
                                stop=True,
                            )
                            attp = ap_.tile([128, 512], BF16, tag="attp")
                            nc.vector.tensor_copy(attp[:], ps[:])
                            bp = bpp.tile([128, 1152], BF16, name=f"bp{tb}", tag=f"bp{tb}", bufs=2)
                            pwriters = _expand_window(
                                nc, _PLANS2, tb, bp, attp[:], mu2,
                                nc.scalar.copy, nc.vector.tensor_copy,
                            )
                            # WAR vs previous unit's shifted reads of this tag
                            for prev in bp_hist[tb]:
                                for wri in pwriters:
                                    add_dep_helper(wri.ins, prev.ins, sync=True, reason="bp WAR")
                            bp_hist[tb] = []
                            bp_gathers[tb] = pwriters
                            bp_tiles[tb] = bp
                            p2 = p2p.tile(
                                [128, 1024], BF16, name=f"p2_{tb}", tag=f"p2_{tb}"
                            )
                            bv = bp[:]
                            sh2 = nc.sync.dma_start(
                                p2[:],
                                bv.__replace__(
                                    offset=bv.offset + 127,
                                    ap=[[1151, 128], [1, 1024]],
                                ),
                            )
                            for wri in pwriters:
                                add_dep_helper(
                                    sh2.ins, wri.ins, sync=True, reason="bp RAW"
                                )
                            bp_hist[tb].append(sh2)
                            p2_tiles[tb] = p2
                        p2 = p2_tiles[tb]
                        ps_sc = psS.tile([128, 512], F32, tag="pssc")
                        for sbl in range(4):
                            # transpose c2p via plain matmul against identity:
                            # out[m,n] = sum_s c2p[s, t_m] I[s, n] = c2pT.
                            # start=True clears the WHOLE PSUM bank, so only
                            # the first one starts the group.
                            nc.tensor.matmul(
                                ps_sc[:, sbl * 128 : (sbl + 1) * 128],
                                c2p_t[sbl][:, tb * 128 : (tb + 1) * 128],
                                identb[:],
                                start=(sbl == 0),
                                stop=False,
                                skip_group_check=True,
                            )
                        nc.tensor.matmul(
                            ps_sc[:],
                            kT[hb][hr : hr + 64, tb * 128 : (tb + 1) * 128],
                            qTs[hb][hr : hr + 64, s0 : s0 + 512],
                            start=False,
                            stop=False,
                        )
                        nc.tensor.matmul(
                            ps_sc[:],
                            identb[:],
                            p2[:, s0 : s0 + 512],
                            start=False,
                            stop=True,
                        )
                        E = ep.tile([128, 512], F32R, tag="E")
                        nc.scalar.activation(E[:], ps_sc[:], AF.Exp)
                        nc.sync.dma_start(
                            t["probsT_out"][
                                h, tb * 128 : (tb + 1) * 128, s0 : s0 + 512
                            ],
                            E[:].bitcast(F32),
                        )
                        nc.tensor.matmul(
                            ps_ctx[:],
                            vaug[tb][:, h * 65 : (h + 1) * 65],
                            E[:],
                            start=(tb == 0),
                            stop=(tb == 7),
                        )
                    zsb = smp.tile([1, 512], F32, tag="zsb")
                    nc.vector.tensor_copy(zsb[:], ps_ctx[64:65, :])
                    nc.sync.dma_start(t["z_out"][h, s0 : s0 + 512], zsb[:])
                    rz = smp.tile([1, 512], F32, tag="rz")
                    nc.vector.reciprocal(rz[:], ps_ctx[64:65, :])
                    rzb = smp.tile([64, 512], F32, tag="rzb")
                    nc.gpsimd.partition_broadcast(rzb[:], rz[:])
                    nc.vector.scalar_tensor_tensor(
                        ctxT[hb][hr : hr + 64, s0 : s0 + 512],
                        ps_ctx[0:64, :],
                        1.0,
                        rzb[:],
                        OP.mult,
                        OP.mult,
                    )

        # ---------------- output projection + AllReduce + LN ----------------
        cc_in = nc.dram_tensor("cc_in", [1024, 768], F32, kind="Internal").ap()
        cc_out = nc.dram_tensor("cc_out", [512, 768], F32, kind="Internal").ap()
        with (
            tc.tile_pool(name="oproj", bufs=3) as op_,
            tc.tile_pool(name="psO", bufs=2, space="PSUM") as psO,
        ):
            for sb in range(8):
                pso = psO.tile([128, 768], F32, tag="pso")
                for kc3 in range(3):
                    for ns, nw in ((0, 512), (512, 256)):
                        nc.tensor.matmul(
                            pso[:, ns : ns + nw],
                            ctxT[kc3][:, sb * 128 : (sb + 1) * 128],
                            woT[kc3][:, ns : ns + nw],
                            start=(kc3 == 0),
                            stop=(kc3 == 2),
                        )
                osb = op_.tile([128, 768], F32, tag="osb")
                nc.vector.tensor_copy(osb[:], pso[:])
                nc.sync.dma_start(cc_in[sb * 128 : (sb + 1) * 128, :], osb[:])

            if SINGLE_CORE_TIMING:
                # stand-in with comparable data movement for the cost model
                nc.sync.dma_start(cc_out[:], cc_in[0:512, :])
            else:
                nc.gpsimd.collective_compute(
                    "ReduceScatter",
                    mybir.AluOpType.add,
                    [[0, 1], [2, 3], [4, 5], [6, 7]],
                    ins=[cc_in[:]],
                    outs=[cc_out[:]],
                )

            lnw = pp.tile([128, 768], F32)
            lnb = pp.tile([128, 768], F32)
            nc.sync.dma_start(lnw[:], t["lnw_b"][:])
            nc.sync.dma_start(lnb[:], t["lnb_b"][:])
            for blk in range(4):
                csb = op_.tile([128, 768], F32, tag="csb")
                nc.sync.dma_start(csb[:], cc_out[blk * 128 : (blk + 1) * 128, :])
                xbo = op_.tile([128, 768], F32, tag="xbo")
                nc.sync.dma_start(
                    xbo[:], t["xbo_half"][blk * 128 : (blk + 1) * 128, :]
                )
                Ot = op_.tile([128, 768], F32, tag="Ot")
                nc.vector.scalar_tensor_tensor(
                    Ot[:], csb[:], 1.0, xbo[:], OP.mult, OP.add
                )
                musum = op_.tile([128, 1], F32, tag="musum")
                nc.vector.tensor_reduce(
                    musum[:], Ot[:], mybir.AxisListType.X, OP.add
                )
                negmu = op_.tile([128, 1], F32, tag="negmu")
                nc.vector.tensor_scalar(
                    negmu[:], musum[:], -1.0 / 768.0, None, OP.mult
                )
                t1 = op_.tile([128, 768], F32, tag="t1")
                nc.scalar.activation(t1[:], Ot[:], AF.Identity, bias=negmu[:])
                sq = op_.tile([128, 768], F32, tag="sq")
                vsum = op_.tile([128, 1], F32, tag="vsum")
                nc.scalar.activation(sq[:], t1[:], AF.Square, accum_out=vsum[:])
                veps = op_.tile([128, 1], F32, tag="veps")
                nc.vector.tensor_scalar(
                    veps[:], vsum[:], 1.0 / 768.0, LN_EPS, OP.mult, OP.add
                )
                sd = op_.tile([128, 1], F32, tag="sd")
                nc.scalar.sqrt(sd[:], veps[:])
                rstd = op_.tile([128, 1], F32, tag="rstd")
                nc.vector.reciprocal(rstd[:], sd[:])
                o1 = op_.tile([128, 768], F32, tag="o1")
                nc.vector.scalar_tensor_tensor(
                    o1[:], t1[:], rstd[:], lnw[:], OP.mult, OP.mult
                )
                o2 = op_.tile([128, 768], F32, tag="o2")
                nc.vector.scalar_tensor_tensor(
                    o2[:], o1[:], 1.0, lnb[:], OP.mult, OP.add
                )
                nc.sync.dma_start(
                    t["out_half"][blk * 128 : (blk + 1) * 128, :], o2[:]
                )


_PROGRAM = {}


def _build_program():
    key = (SINGLE_CORE_TIMING, DEBUG_DUMPS)
    if key in _PROGRAM:
        return _PROGRAM[key]
    nc = bacc.Bacc(
        "TRN2",
        target_bir_lowering=False,
        debug=False,
        num_devices=1 if SINGLE_CORE_TIMING else 8,
    )
    t = {}

    def inp(name, shape, dtype=F32):
        t[name] = nc.dram_tensor(name, shape, dtype, kind="ExternalInput").ap()

    inp("xT_aug", [128, 7168])
    inp("WqTs_aug", [128, 2688])
    inp("WkT_aug", [128, 2688])
    inp("WvT_aug", [128, 2688])
    inp("pos_embT_aug", [128, 3584])
    inp("WoT_slice", [384, 768])
    inp("xbo_half", [512, 768])
    inp("lnw_b", [128, 768])
    inp("lnb_b", [128, 768])
    inp("ident_f32", [128, 128])
    inp("bq_c", [128, 3])
    inp("bk_c", [128, 3])
    inp("mu1_w", [128, _MU1W.shape[1]], U16)
    inp("mu2_w", [128, _MU2W.shape[1]], U16)
    t["probsT_out"] = nc.dram_tensor(
        "probsT_out", [NH, 1024, 1024], F32, kind="ExternalOutput"
    ).ap()
    t["z_out"] = nc.dram_tensor("z_out", [NH, 1024], F32, kind="ExternalOutput").ap()
    t["out_half"] = nc.dram_tensor(
        "out_half", [512, 768], F32, kind="ExternalOutput"
    ).ap()
    if DEBUG_DUMPS:
        t["attc_dbg"] = nc.dram_tensor(
            "attc_dbg", [8, 128, 512], F32, kind="ExternalOutput"
        ).ap()
        t["bc_dbg"] = nc.dram_tensor(
            "bc_dbg", [8, 128, 1152], F32, kind="ExternalOutput"
        ).ap()
        t["c2p_dbg"] = nc.dram_tensor(
            "c2p_dbg", [8, 128, 1024], F32, kind="ExternalOutput"
        ).ap()

    with tile.TileContext(nc) as tc:
        _emit(tc, t)
    nc.compile()
    _PROGRAM[key] = (nc, t)
    return _PROGRAM[key]


def _host_inputs(inputs):
    hidden = np.ascontiguousarray(np.asarray(inputs["hidden_states"], np.float32))
    rel_emb = np.asarray(inputs["rel_embeddings"], np.float32)
    Wq = np.asarray(inputs["Wq"], np.float32)
    bq = np.asarray(inputs["bq"], np.float32)
    Wk = np.asarray(inputs["Wk"], np.float32)
    bk = np.asarray(inputs["bk"], np.float32)
    Wv = np.asarray(inputs["Wv"], np.float32)
    bv = np.asarray(inputs["bv"], np.float32)
    Wo = np.asarray(inputs["Wo"], np.float32)
    bo = np.asarray(inputs["bo"], np.float32)
    ln_w = np.asarray(inputs["ln_w"], np.float32)
    ln_b = np.asarray(inputs["ln_b"], np.float32)

    mu1_w, mu2_w = _MU1W, _MU2W
    ident = np.eye(128, dtype=np.float32)
    lnw_b = np.ascontiguousarray(np.broadcast_to(ln_w, (128, 768)))
    lnb_b = np.ascontiguousarray(np.broadcast_to(ln_b, (128, 768)))
    posT_aug_packed = None  # built below via aug()

    def aug(mat, bias):
        full = np.concatenate([mat, bias[None, :]], axis=0).astype(np.float32)
        cols = full.shape[1]
        packed = np.zeros((128, 7 * cols), np.float32)
        for c in range(6):
            packed[:, c * cols : (c + 1) * cols] = full[c * 128 : (c + 1) * 128]
        packed[0, 6 * cols : 7 * cols] = full[768]
        return packed

    posT_aug_packed = aug(rel_emb[:NB].T.astype(np.float32), np.ones(NB, np.float32))
    in_maps = []
    for c in range(8):
        b, hh = c // 2, c % 2
        sl = slice(hh * NH * D, (hh + 1) * NH * D)
        in_maps.append(
            {
                "xT_aug": aug(hidden[b].T, np.ones(1024, np.float32)),
                "WqTs_aug": aug(Wq[sl].T * SCALE, bq[sl] * SCALE),
                "WkT_aug": aug(Wk[sl].T, bk[sl]),
                "WvT_aug": aug(Wv[sl].T, bv[sl]),
                "pos_embT_aug": posT_aug_packed,
                "WoT_slice": np.ascontiguousarray(Wo[:, sl].T),
                "xbo_half": np.ascontiguousarray(
                    hidden[b, hh * 512 : (hh + 1) * 512] + bo
                ),
                "lnw_b": lnw_b,
                "lnb_b": lnb_b,
                "ident_f32": ident,
                "bq_c": np.ascontiguousarray(
                    (bq[sl] * SCALE).reshape(3, 128).T
                ),
                "bk_c": np.ascontiguousarray(bk[sl].reshape(3, 128).T),
                "mu1_w": mu1_w,
                "mu2_w": mu2_w,
            }
        )
    return in_maps


def kernel(**inputs):
    in_maps = _host_inputs(inputs)
    nc, _ = _build_program()
    res = bass_utils.run_bass_kernel_spmd(nc, in_maps, core_ids=list(range(8)))
    rs = res.results
    out = np.zeros((B, S, H), np.float32)
    probs = np.zeros((B, HEADS, S, S), np.float32)
    for c in range(8):
        b, hh = c // 2, c % 2
        out[b, hh * 512 : (hh + 1) * 512] = rs[c]["out_half"]
        pT = rs[c]["probsT_out"]  # [6, t, s] unnormalized
        Z = rs[c]["z_out"]  # [6, s]
        probs[b, hh * NH : (hh + 1) * NH] = np.swapaxes(pT, 1, 2) / Z[:, :, None]
    return out, probs
